# revision 9
# baseline (speedup 1.0000x reference)
"""Trainium2 Bass kernel for Conf-MPU loss (nn_Conf_MPULoss) — v4.

Host side: rows sorted by label t into 5 class groups, split evenly across 8
cores, each per-core class segment padded to S = 128*R rows with sentinel rows
(non-label logits -10, label-class +10 pattern; exact in bf16). x ships as
bf16 PLANAR per segment: [P, 6 planes, R]. For segment c<4 the plane order is
[j0, j1, j2, x4, x_c, -x_c] (j = the non-c classes ascending, so plane 3 is
always the negative-class logit and plane 5 is pre-negated x_c so a single
contiguous ScalarE exp yields 1/e_c). Segment 4 uses natural order, plane 5
unused (not transferred).

Device per class segment c (planes as [P, R] bf16 slices of E = exp(X)):
    exp  : two ScalarE instrs (planes 0:3, 3:6) -> e0..e3, e_c, 1/e_c
    pa1  = e0 + e2 ; pa2 = e1 + e3           Pool (GpSimd) adds
    zp   = pa1 + pa2  (= sum of non-c exps)  DVE 2x TT
    z    = zp + e_c                          DVE
    lnz  = ln(z)                             ScalarE
    m    = (e_c > zp)  (== p_c > 1/2)        DVE is_gt TT
    d4   = lnz - x4    (= -log p_neg)        DVE
    q    = d4 * z ; u = q * (1/e_c)          DVE  (u = -log(p4)/p_c)
    g    = m * u                             DVE
    c==4: max-tree over e-planes (DVE), mn = (2*max <= z) STT, g = mn*d4
Per-class sums (den=sum m, num=sum g, li=sum g4) are colsum-matmuls with a
ones vector on the otherwise-idle PE into PSUM rows, extracted once at the
end by a single DVE tensor_scalar accum -> [9,1] f32 -> one tiny DMA out.
risk1-risk3 needs only sum(x4-x_c) over rows with t=c, computed exactly on
the host (f64) during packing. Host all-reduces the 9-vector across cores and
does the final scalar combination.

Emission is software-pipelined across the 5 segments (seg4 first, seg3 last
with the final ln/product chain in half-chunks to shorten the tail). exp
without max-subtraction is bf16-safe: logits are O(1), pads give exact zeros
in every masked accumulator.
"""

import ml_dtypes
import numpy as np

import concourse.bacc as bacc
import concourse.mybir as mybir
import concourse.tile as tile
from concourse import bass_utils

F32 = mybir.dt.float32
BF16 = mybir.dt.bfloat16
Alu = mybir.AluOpType
Act = mybir.ActivationFunctionType

P = 128
NCLS = 5
N_CORES = 8
# stat rows: den c -> c (c<4), num c -> 4+c, li -> 8
NSTAT = 9
PSW = 512  # psum bank free width (f32)

_PROGRAM_CACHE: dict[int, tuple] = {}


def _restrict_act_tables(arch: str):
    """Confine Exp/Ln to the natural_log_exp_and_others set so the act-table
    pass emits a single ACT_TABLE_LOAD instead of thrashing between the
    exp_and_others and natural_log sets (~1.3us per load)."""
    from concourse import hw_specs

    tables = hw_specs.get_activation_tables(arch)
    if "natural_log_exp_and_others" not in tables:
        return
    for name, funcs in tables.items():
        if name != "natural_log_exp_and_others":
            funcs.discard(Act.Exp)
            funcs.discard(Act.Ln)


def _build_program(R: int):
    """Build + compile the per-core Bass program for segment length S=128*R."""
    nc = bacc.Bacc("TRN2", debug=False, num_devices=N_CORES)
    _restrict_act_tables(nc.m.arch)
    x_d = nc.dram_tensor("x", [NCLS, P, 6 * R], BF16, kind="ExternalInput").ap()
    st_d = nc.dram_tensor("stats", [NSTAT, 1], F32, kind="ExternalOutput").ap()

    with tile.TileContext(nc) as tc:
        with (
            tc.tile_pool(name="io", bufs=1) as iop,
            tc.tile_pool(name="ep", bufs=1) as epool,
            tc.tile_pool(name="wk", bufs=1) as wp,
            tc.tile_pool(name="st", bufs=1) as sp,
            tc.tile_pool(name="ps", bufs=1, space="PSUM") as pp,
        ):
            # per-stat one-hot weight columns: W_s = wones[:, 9s:9s+9] has ones
            # only in column s, so matmul adds colsums into psum row s only.
            wones = sp.tile([P, NSTAT * NSTAT], BF16)
            psum = pp.tile([NSTAT, PSW], F32)
            ext = sp.tile([NSTAT, PSW], F32)
            stats = sp.tile([NSTAT, 1], F32)
            nc.vector.memset(wones, 0.0)
            for s_ in range(NSTAT):
                nc.vector.memset(wones[:, NSTAT * s_ + s_ : NSTAT * s_ + s_ + 1], 1.0)

            X = {}
            E = {}
            seg = {}

            def alloc(c):
                n = 6 if c < 4 else 5
                X[c] = iop.tile([P, n * R], BF16, tag=f"x{c}", name=f"x{c}")
                E[c] = epool.tile([P, n * R], BF16, tag=f"e{c}", name=f"e{c}")
                s = {}
                names = ("a", "zp", "z", "lnz", "d4", "m", "g")
                if c < 4:
                    names += ("ze5", "u")
                else:
                    names += ("mm", "m3", "mx")
                for t in names:
                    w = 2 * R if t in ("a", "mm") else R
                    s[t] = wp.tile([P, w], BF16, tag=f"{t}_{c}", name=f"{t}_{c}")
                seg[c] = s

            for c in range(NCLS):
                alloc(c)

            def dma_in(c, jlo, jhi, eng):
                eng.dma_start(
                    out=X[c][:, jlo * R : jhi * R], in_=x_d[c][:, jlo * R : jhi * R]
                )

            def exp(c, jlo, jhi):
                nc.scalar.activation(
                    E[c][:, jlo * R : jhi * R], X[c][:, jlo * R : jhi * R], Act.Exp
                )

            def ln(c, lo=0, hi=None):
                hi = R if hi is None else hi
                s = seg[c]
                nc.scalar.activation(s["lnz"][:, lo:hi], s["z"][:, lo:hi], Act.Ln)

            def pair(c):
                # a = [e0+e2 | e1+e3] as one 2R-wide 2x TT (planes 0:4)
                s = seg[c]
                nc.vector.tensor_tensor(
                    out=s["a"], in0=E[c][:, 0 : 2 * R], in1=E[c][:, 2 * R : 4 * R],
                    op=Alu.add,
                )

            def max_pair(c):
                s = seg[c]
                nc.vector.tensor_tensor(
                    out=s["mm"], in0=E[c][:, 0 : 2 * R], in1=E[c][:, 2 * R : 4 * R],
                    op=Alu.max,
                )

            def zsum(c):
                # zp = pa1 + pa2 (sum of the 4 non-c exps; needs planes 0:4)
                s = seg[c]
                nc.vector.tensor_tensor(
                    out=s["zp"], in0=s["a"][:, 0:R], in1=s["a"][:, R : 2 * R], op=Alu.add
                )

            def zfin(c):
                # z = zp + e_c ; (c<4) m = e_c > zp ; ze5 = z/e_c (pre-ln)
                s = seg[c]
                ec = E[c][:, 4 * R : 5 * R]
                nc.vector.tensor_tensor(out=s["z"], in0=s["zp"], in1=ec, op=Alu.add)
                if c < 4:
                    nc.vector.tensor_tensor(
                        out=s["m"], in0=ec, in1=s["zp"], op=Alu.is_gt
                    )
                    nc.vector.tensor_tensor(
                        out=s["ze5"], in0=s["z"], in1=E[c][:, 5 * R : 6 * R],
                        op=Alu.mult,
                    )

            def maxtree4a():
                s = seg[4]
                nc.vector.tensor_tensor(
                    out=s["m3"], in0=s["mm"][:, 0:R], in1=s["mm"][:, R : 2 * R],
                    op=Alu.max,
                )

            def maxtree4b():
                s = seg[4]
                nc.vector.tensor_tensor(
                    out=s["mx"], in0=s["m3"], in1=E[4][:, 4 * R : 5 * R], op=Alu.max
                )

            def grp(c, lo=0, hi=None):
                # post-ln product chain
                hi = R if hi is None else hi
                s = seg[c]
                w = lambda t: s[t][:, lo:hi]
                if c < 4:
                    # d4 = lnz - x4 (plane 3); u = d4 * z/e_c; g = m * u
                    nc.vector.tensor_tensor(
                        out=w("d4"), in0=w("lnz"),
                        in1=X[c][:, 3 * R + lo : 3 * R + hi], op=Alu.subtract,
                    )
                    nc.vector.tensor_tensor(
                        out=w("u"), in0=w("d4"), in1=w("ze5"), op=Alu.mult
                    )
                    nc.vector.tensor_tensor(
                        out=w("g"), in0=w("m"), in1=w("u"), op=Alu.mult
                    )
                else:
                    # d4 = lnz - x4 (plane 4); mn = (2*mx <= z); g = mn * d4
                    nc.vector.tensor_tensor(
                        out=w("d4"), in0=w("lnz"),
                        in1=X[c][:, 4 * R + lo : 4 * R + hi], op=Alu.subtract,
                    )
                    nc.vector.scalar_tensor_tensor(
                        out=w("m"), in0=w("mx"), scalar=2.0, in1=w("z"),
                        op0=Alu.mult, op1=Alu.is_le,
                    )
                    nc.vector.tensor_tensor(
                        out=w("g"), in0=w("m"), in1=w("d4"), op=Alu.mult
                    )

            def colsum(row, src, lo, hi, first, last):
                # psum[row] += per-column sums of src[:, lo:hi] via a one-hot
                # ones-column matmul. All stats share one accumulation group
                # on the [NSTAT, PSW] region; `first`/`last` only for the very
                # first/last matmul overall.
                chunks = []
                a = lo
                while a < hi:
                    b = min(a + PSW, hi)
                    chunks.append((a, b))
                    a = b
                for i, (a, b) in enumerate(chunks):
                    nc.tensor.matmul(
                        out=psum[:, 0 : b - a],
                        lhsT=wones[:, NSTAT * row : NSTAT * row + NSTAT],
                        rhs=src[:, a:b],
                        start=(first and i == 0),
                        stop=(last and i == len(chunks) - 1),
                        skip_group_check=True,
                    )

            h = R // 2
            q3 = (3 * R // 4 + 1) // 2 * 2  # seg3 split point (even)
            # DMA: all on the sync HWDGE ring in strict compute order; seg4
            # in small chunks so the first exp starts ASAP, seg0 split for
            # pipelining, segs 1-3 as single transfers (DMA runs well ahead).
            dma_in(4, 0, 2, nc.sync)
            dma_in(4, 2, 4, nc.sync)
            dma_in(4, 4, 5, nc.sync)
            dma_in(0, 0, 4, nc.sync)
            dma_in(0, 4, 6, nc.sync)
            dma_in(1, 0, 6, nc.sync)
            dma_in(2, 0, 6, nc.sync)
            dma_in(3, 0, 6, nc.sync)

            # ---- software-pipelined emission ----
            exp(4, 0, 2)
            exp(4, 2, 4)
            pair(4)
            max_pair(4)
            zsum(4)
            maxtree4a()
            exp(4, 4, 5)
            zfin(4)
            maxtree4b()
            exp(0, 0, 4)
            ln(4)
            pair(0)
            zsum(0)
            grp(4)
            colsum(8, seg[4]["g"], 0, R, True, False)  # li (opens psum group)
            exp(0, 4, 6)
            zfin(0)
            exp(1, 0, 6)
            ln(0)
            colsum(0, seg[0]["m"], 0, R, False, False)  # den0
            pair(1)
            zsum(1)
            grp(0)
            colsum(4, seg[0]["g"], 0, R, False, False)  # num0
            zfin(1)
            exp(2, 0, 6)
            ln(1)
            colsum(1, seg[1]["m"], 0, R, False, False)
            pair(2)
            zsum(2)
            grp(1)
            colsum(5, seg[1]["g"], 0, R, False, False)
            zfin(2)
            exp(3, 0, 6)
            ln(2)
            colsum(2, seg[2]["m"], 0, R, False, False)
            pair(3)
            zsum(3)
            grp(2)
            colsum(6, seg[2]["g"], 0, R, False, False)
            zfin(3)
            ln(3, 0, q3)
            colsum(3, seg[3]["m"], 0, R, False, False)
            grp(3, 0, q3)
            ln(3, q3, R)
            colsum(7, seg[3]["g"], 0, q3, False, False)
            grp(3, q3, R)
            colsum(7, seg[3]["g"], q3, R, False, True)
            # extract all psum rows -> [NSTAT,1] f32, then one tiny DMA out
            nc.vector.tensor_scalar(
                out=ext,
                in0=psum,
                scalar1=1.0,
                scalar2=0.0,
                op0=Alu.mult,
                op1=Alu.add,
                accum_out=stats,
            )
            nc.sync.dma_start(out=st_d, in_=stats)
    nc.compile()
    return nc


def _get_program(R: int):
    if R not in _PROGRAM_CACHE:
        _PROGRAM_CACHE[R] = _build_program(R)
    return _PROGRAM_CACHE[R]


def _prepare_inputs(x: np.ndarray, t: np.ndarray):
    """Sort rows by class, shard across cores, pad segments, pack planar bf16
    with per-segment plane permutation + negated-label plane. Also computes
    the exact host-side per-class sum(x4 - xc) (risk1-risk3 accumulator).
    Returns (in_maps, counts, sd, R)."""
    N = x.shape[0]
    t64 = t.astype(np.int64, copy=False)
    counts = np.bincount(t64, minlength=NCLS).astype(np.int64)

    n_ck = np.zeros((NCLS, N_CORES), dtype=np.int64)
    for c in range(NCLS):
        q, r = divmod(int(counts[c]), N_CORES)
        n_ck[c] = q
        n_ck[c, :r] += 1

    R = int(max(8, -(-int(n_ck.max()) // P)))
    R = (R + 1) // 2 * 2  # keep it even
    S = P * R

    order = np.argsort(t64, kind="stable")
    xs = np.ascontiguousarray(x[order], dtype=np.float32)
    starts = np.concatenate([[0], np.cumsum(counts)])

    # host-exact sum(x4 - xc) per positive class
    sd = np.zeros(4, dtype=np.float64)
    for c in range(4):
        blk = xs[int(starts[c]) : int(starts[c + 1])]
        sd[c] = blk[:, 4].astype(np.float64).sum() - blk[:, c].astype(np.float64).sum()

    # planar layout per (core, segment): [P, 6 planes, R]
    xcores = np.empty((N_CORES, NCLS, P, 6, R), dtype=np.float32)
    for c in range(NCLS):
        if c < 4:
            cols = [j for j in range(5) if j != c] + [c]
            padv = np.array([-10.0] * 3 + [10.0, -10.0, 10.0], dtype=np.float32)
        else:
            cols = [0, 1, 2, 3, 4]
            padv = np.array([-10.0] * 4 + [10.0, 0.0], dtype=np.float32)
        off = int(starts[c])
        for k in range(N_CORES):
            n = int(n_ck[c, k])
            blk = np.empty((S, 6), dtype=np.float32)
            if n:
                blk[:n, :5] = xs[off : off + n][:, cols]
                blk[:n, 5] = -blk[:n, 4] if c < 4 else 0.0
            blk[n:] = padv
            xcores[k, c] = blk.reshape(P, R, 6).transpose(0, 2, 1)
            off += n

    xb = xcores.reshape(N_CORES, NCLS, P, 6 * R).astype(ml_dtypes.bfloat16)
    in_maps = [{"x": xb[k]} for k in range(N_CORES)]
    return in_maps, counts, sd, R


def _combine(stats_list, counts, sd, N):
    """Host all-reduce of the per-class accumulators + final scalar combination."""
    st = np.zeros(NSTAT, dtype=np.float64)
    for s in stats_list:
        st += s.astype(np.float64).reshape(-1)

    counts = counts.astype(np.float64)
    r13 = 0.0  # risk1 - risk3
    r2 = 0.0
    for c in range(4):
        den = st[c]
        num = st[4 + c]
        prior = counts[c] / N
        r13 += prior * sd[c] / max(1.0, counts[c])
        r2 += prior * num / max(den, 1.0)
    r4 = st[8] / max(1.0, counts[4])

    pos = 4.0 * (r13 + r2)
    if pos < 0.0:
        pos = 0.0
    return np.float32(pos + r4)


def run_device(in_maps, R, trace=False, **kw):
    nc = _get_program(R)
    res = bass_utils.run_bass_kernel_spmd(
        nc, in_maps, core_ids=list(range(N_CORES)), trace=trace, **kw
    )
    return res


def kernel(x: np.ndarray, t: np.ndarray) -> np.ndarray:
    x = np.asarray(x, dtype=np.float32)
    t = np.asarray(t)
    N = x.shape[0]
    in_maps, counts, sd, R = _prepare_inputs(x, t)
    res = run_device(in_maps, R)
    stats_list = [res.results[k]["stats"] for k in range(N_CORES)]
    return _combine(stats_list, counts, sd, N)
